# revision 29
# baseline (speedup 1.0000x reference)
"""Trainium2 Bass kernel for nn_PamCell (spatial self-attention, B=4, C=64,
N=16^3=4096, CQ=8) on 8 NeuronCores.

Sharding: core i handles batch i//2 and query-half i%2 (2048 queries vs all
4096 keys). No collectives; host scatters inputs / gathers outputs.

Math: softmax rows are invariant to additive terms that depend only on the
query index, so with A = wq^T wk and u = wk^T bq,
    softmax(q k^T)[n, :] == softmax((A^T x_n + u) . x_m)[n, :]
which turns the QK contraction into a 64-dim contraction against the raw
input as keys. A is scaled by S=16 on the host (fp8e4m3 would denormalize
its ~0.007-magnitude entries) and descaled inside the exp.

Per-core pipeline (2 query-phases of 1024; all energies fp8 DoubleRow):
  prologue: Q = a^T x in fp8-DR writing channel-pair layout psum, +u and
            fp8 cast on DVE; v^T per key chunk in fp8-DR, bias+copy on Pool.
  loop over 32 key chunks per phase:
      e[128,1024] = two fp8-DR matmuls (row bands 0/32)      (PE)
      p = exp(e/S): 3 of 4 chunks on ACT; every 4th on DVE as a
          Schraudolph exp2 bit-trick (int16 affine, bitcast bf16)
      out[65,1024] += [v^T|1]^T @ p  (row 64 = denominator)  (PE, bf16)
  epilogue (overlapped into next phase): r = 1/den (DVE), broadcast to 64
  partitions via f32r ones-matmul, out*r (DVE) + x (Pool), bf16 DMA out.
"""

import sys

import numpy as np

try:
    import concourse.bass as bass
except ImportError:  # fresh interpreter without the env paths
    for _p in ("/root/.axon_site", "/root/.axon_site/_ro/trn_rl_repo",
               "/root/.axon_site/_ro/pypackages", "/opt/trn_rl_repo"):
        if _p not in sys.path:
            sys.path.append(_p)
    import concourse.bass as bass

import ml_dtypes

import concourse.tile as tile
from concourse import mybir
from concourse.vector_clock import ScopedClock

BF16 = mybir.dt.bfloat16
F32 = mybir.dt.float32
F32R = mybir.dt.float32r
F8 = mybir.dt.float8e4
I16 = mybir.dt.int16
AF = mybir.ActivationFunctionType
DR = mybir.MatmulPerfMode.DoubleRow

B, C, N = 4, 64, 4096
NQ = N // 2          # queries per core
NKC = N // 128       # key chunks of 128
NQP = 1024           # queries per phase
N_CORES = 8

ESCALE = 16.0                       # host scale on A/u vs fp8 denormals
LOG2E = 1.4426950408889634
SCHRA_SCALE = 128.0 * LOG2E / ESCALE
SCHRA_BIAS = 16256.0
DVE_CHUNK = lambda c: (c % 4) == 3  # exp-offload set (8 of 32 per phase)


class _TileContextCompat(tile.TileContext):
    """Split the kernel-tail drain's sem waits across SP instructions;
    this walrus build allows only one sync-wait per CTRL instruction."""

    def _drain_and_barrier(self, tick_clock, wait_clock):
        probe = self.nc.sync.nop()
        wait_clock.add_sem_waits(
            probe.ins, ScopedClock({None: tick_clock.global_clock})
        )
        si = probe.ins.sync_info
        waits = list(si.on_wait) if si is not None else []
        if si is not None:
            probe.ins.sync_info = mybir.SyncInfo(
                on_wait=waits[:1], on_update=list(si.on_update)
            )
        for w in waits[1:]:
            nop = self.nc.sync.nop()
            nop.ins.sync_info = mybir.SyncInfo(on_wait=[w], on_update=[])

        self.nc.sync.drain()
        self.nc.all_engine_barrier()
        assert self.sems is not None
        popped = self.nc._tile_sem_poison_stack.pop()
        assert popped is self._sem_poison
        self.nc.clear_and_free_semaphores(list(self.sems.allocated().values()))
        self.nc.all_engine_barrier()


def _split_sync_waits(nc, max_waits=1):
    """This walrus build rejects instructions carrying more than one sync
    wait; hoist excess waits onto same-engine nops inserted just before."""
    for fn in nc.m.functions:
        for blk in fn.blocks:
            new = []
            changed = False
            for inst in blk.instructions:
                si = inst.sync_info
                if si is not None and si.on_wait and len(si.on_wait) > max_waits:
                    waits = list(si.on_wait)
                    excess = waits[:-max_waits]
                    for i in range(0, len(excess), max_waits):
                        nop = mybir.InstNoOp(
                            name=f"I-{nc.next_id()}-waitsplit", ins=[], outs=[]
                        )
                        nop.engine = inst.engine
                        nop.sync_info = mybir.SyncInfo(
                            on_wait=excess[i : i + max_waits], on_update=[]
                        )
                        new.append(nop)
                    inst.sync_info = mybir.SyncInfo(
                        on_wait=waits[-max_waits:], on_update=list(si.on_update)
                    )
                    changed = True
                new.append(inst)
            if changed:
                blk.instructions = new


def build_nc(split=True):
    nc = bass.Bass(
        "TRN2",
        target_bir_lowering=False,
        debug=False,
        enable_asserts=False,
    )
    # [32b+p, c, s, m] = fp8(x[2p+s, 128c+m]); bands b=0,1 identical copies
    xk_f8 = nc.dram_tensor("xk_f8", (C, NKC, 2, 128), F8, kind="ExternalInput")
    # [p, i, s, mp] = fp8(S*A[2p+i, 2mp+s])
    a_dr = nc.dram_tensor("a_dr", (32, 2, 2, 32), F8, kind="ExternalInput")
    # [p, s] = S*u[2p+s]
    u_sc = nc.dram_tensor("u_sc", (32, 2), F32, kind="ExternalInput")
    # [p, i, c] = fp8(gamma*wv[c, 2p+i])
    wv_dr = nc.dram_tensor("wv_dr", (32, 2, C), F8, kind="ExternalInput")
    # gamma*bv as a per-partition column; bv normalizes out of the softmax
    # ratio (bv*den/den), so it is added post-division in the epilogue.
    bv_col = nc.dram_tensor("bv_col", (C, 1), F32, kind="ExternalInput")
    xq_res = nc.dram_tensor("xq_res", (C, NQ), BF16, kind="ExternalInput")
    out = nc.dram_tensor("out", (C, NQ), BF16, kind="ExternalOutput")

    with _TileContextCompat(nc) as tc:
        with tc.tile_pool(name="consts", bufs=1) as consts:
            xk8 = consts.tile([C, NKC, 2, 128], F8, tag="xk8")
            a_sb = consts.tile([32, 2, 2, 32], F8, tag="a_sb")
            u_sb = consts.tile([32, 2], F32, tag="u_sb")
            qstage = consts.tile([32, NQ], F8, tag="qstage")
            wv_sb = consts.tile([32, 2, C], F8, tag="wv_sb")
            bv_sb = consts.tile([C, 1], F32, tag="bv_sb")
            xq_sb = consts.tile([C, NQ], BF16, tag="xq_sb")
            qb = consts.tile([C, NQ], F8, tag="qb")
            vt = consts.tile([128, NKC, C + 1], BF16, tag="vt")
            rec = consts.tile([1, NQ], BF16, tag="rec")
            ones_f = consts.tile([1, 128], F32, tag="ones_f")
            ones_b = consts.tile([1, 128], BF16, tag="ones_b")
            warm_sb = consts.tile([1, 128], F32, tag="warm_sb")

            import bass_rust as _br

            pe_chain = [None]
            act_chain = [None]
            dve_chain = [None]

            def _chained(r, chain, reason="order"):
                if chain[0] is not None:
                    _br.add_dep_helper(r.ins, chain[0].ins, reason=reason)
                chain[0] = r
                return r

            nc.vector.memset(ones_f[:], 1.0)
            nc.vector.memset(ones_b[:], 1.0)
            nc.gpsimd.memset(vt[:, :, C : C + 1], 1.0)
            # trigger the table load (natural_log set: Ln + Exp + Copy)
            _chained(nc.scalar.activation(warm_sb[:], ones_f[:], AF.Ln), act_chain)

            # ---- input DMAs ----
            # smalls first on sync; fp8 keys split (band, key-half) over the
            # 4 cheap queues with the core's query-half chunks first; the
            # bf16 residual (needed only by the epilogue) last on sync.
            nc.sync.dma_start(a_sb[:], a_dr.ap())
            nc.sync.dma_start(u_sb[:], u_sc.ap())
            nc.sync.dma_start(wv_sb[:], wv_dr.ap())
            nc.sync.dma_start(bv_sb[:], bv_col.ap())
            for band, eng in ((0, nc.sync), (1, nc.gpsimd)):
                for lo, hi in ((0, 8), (8, 16), (16, 32)):
                    eng.dma_start(
                        xk8[bass.ts(band, 32), lo:hi, :, :],
                        xk_f8.ap()[bass.ts(band, 32), lo:hi, :, :],
                    )
            nc.scalar.dma_start(xq_sb[:], xq_res.ap())

            # NOTE: "query-half chunks first" needs the host to order the
            # chunk dim so this core's query window is chunks 0..15; host
            # rolls the chunk axis per core (see host_prep) and the output
            # is written back in rolled order too (queries are chunks 0..7
            # of phase A, 8..15 of phase B).

            # ---- prologue ----
            with tc.tile_pool(name="psum_pro", bufs=1, space="PSUM") as pro:
                # Q in fp8-DR. DR matmuls can only write psum partition base
                # 0, so both query groups land on partitions 0-31; the g=1
                # group is staged and partition-shifted to qb[32:64] with an
                # sbuf->sbuf DMA (only DMA can remap partitions).
                qb_r = qb.rearrange("p (ph s j) -> p ph s j", ph=2, s=2)
                for ph in range(2):
                    for g in range(2):
                        q32 = pro.tile([32, NQP], F32, tag="q32",
                                       name=f"q32_{ph}{g}")
                        qg = 2 * ph + g
                        for s in range(2):
                            for w in range(4):
                                cw = 4 * qg + w  # chunk window (rolled order)
                                _chained(nc.tensor.matmul(
                                    q32[:, bass.ds(512 * s + 128 * w, 128)],
                                    a_sb[:, :, s, :],
                                    xk8[0:32, cw, :, :],
                                    start=True, stop=True, perf_mode=DR,
                                ), pe_chain)
                        # +u then fp8 cast on DVE (scalar AP per partition)
                        dst = (qb if g == 0 else qstage)
                        for s in range(2):
                            _chained(nc.vector.tensor_scalar(
                                dst[0:32, bass.ds(NQP * ph, NQP)].rearrange(
                                    "p (s j) -> p s j", s=2)[:, s, :],
                                q32[:, bass.ts(s, 512)],
                                u_sb[:, s : s + 1], None,
                                op0=mybir.AluOpType.add,
                            ), dve_chain)
                    nc.scalar.dma_start(
                        qb[32:64, bass.ts(ph, NQP)],
                        qstage[0:32, bass.ts(ph, NQP)],
                    )

                # v^T per key chunk, fp8-DR; bf16 copy on DVE (bv is folded
                # into the epilogue instead).
                for grp in range(NKC // 4):
                    vp = pro.tile([128, 4, C], F32, tag="vp", bufs=1, name="vp")
                    for k in range(4):
                        c = 4 * grp + k
                        mm = nc.tensor.matmul(
                            vp[:, k, :],
                            xk8[0:32, c, :, :],
                            wv_sb[:],
                            start=True, stop=True, perf_mode=DR,
                        )
                        if grp < 2:  # early chunks gate the first outs
                            _chained(mm, pe_chain)
                    _chained(nc.vector.tensor_copy(
                        vt[:, bass.ts(grp, 4), :C], vp[:]
                    ), dve_chain)

            # ---- main loop ----
            with (
                tc.tile_pool(name="psum_e", bufs=2, space="PSUM") as pe_pool,
                tc.tile_pool(name="psum_out", bufs=2, space="PSUM") as pout,
                tc.tile_pool(name="ptb_pool", bufs=3) as ptb_pool,
                tc.tile_pool(name="pti_pool", bufs=2) as pti_pool,
                tc.tile_pool(name="epi_pool", bufs=2) as epi_pool,
            ):
                def energy(ph, c):
                    e = pe_pool.tile([128, 1024], F32, tag="e", name=f"e{ph}_{c}")
                    for g in range(2):
                        _chained(nc.tensor.matmul(
                            e[:, bass.ts(g, 512)],
                            xk8[bass.ts(g, 32), c, :, :],
                            qb_r[bass.ts(g, 32), ph, :, :],
                            start=True, stop=True, perf_mode=DR,
                        ), pe_chain, "pe-order")
                    return e

                def do_exp(e, ph, c):
                    if DVE_CHUNK(c):
                        pt = pti_pool.tile([128, 1024], I16, tag="pti",
                                           name=f"pti{ph}_{c}")
                        _chained(nc.vector.tensor_scalar(
                            pt[:], e[:], SCHRA_SCALE, SCHRA_BIAS,
                            op0=mybir.AluOpType.mult, op1=mybir.AluOpType.add,
                        ), dve_chain, "dve-order")
                        return pt, True
                    pt = ptb_pool.tile([128, 1024], BF16, tag="ptb",
                                       name=f"ptb{ph}_{c}")
                    _chained(nc.scalar.activation(
                        pt[:], e[:], AF.Exp, scale=1.0 / ESCALE,
                    ), act_chain, "act-order")
                    return pt, False

                def outs(out_ph, pt, is_i16, ph, c):
                    for g in range(2):
                        rhs = pt[:, bass.ts(g, 512)]
                        if is_i16:
                            rhs = rhs.bitcast(BF16)
                        _chained(nc.tensor.matmul(
                            out_ph[:, bass.ts(g, 512)],
                            vt[:, c, :],
                            rhs,
                            start=(c == 0), stop=(c == NKC - 1),
                            skip_group_check=True,
                        ), pe_chain, "pe-order")

                def epilogue(ph, out_ph):
                    # r = 1/den (bf16) on DVE; broadcast to 64 partitions
                    # with a K=1 ones-matmul into an e-pool psum slot, then
                    # to SBUF on ACT (tensor_tensor cannot read two PSUMs).
                    with nc.allow_low_precision(reason="1/den fits bf16"):
                        _chained(nc.vector.reciprocal(
                            rec[:, bass.ts(ph, NQP)], out_ph[C : C + 1, :]
                        ), dve_chain, "dve-order")
                    bce = pe_pool.tile([128, 1024], F32, tag="e", name=f"bc{ph}")
                    for g in range(2):
                        _chained(nc.tensor.matmul(
                            bce[:C, bass.ts(g, 512)],
                            ones_b[:, :C],
                            rec[:, bass.ds(NQP * ph + 512 * g, 512)],
                            start=True, stop=True, skip_group_check=True,
                        ), pe_chain, "pe-order")
                    bc_sb = epi_pool.tile([C, NQP], F32, tag="bc_sb",
                                          name=f"bc_sb{ph}")
                    _chained(nc.scalar.copy(bc_sb[:], bce[:C, :]),
                             act_chain, "act-order")
                    tm = epi_pool.tile([C, NQP], F32, tag="tm", name=f"tm{ph}")
                    _chained(nc.vector.tensor_tensor(
                        tm[:], out_ph[:C, :], bc_sb[:], mybir.AluOpType.mult,
                    ), dve_chain, "dve-order")
                    t2 = epi_pool.tile([C, NQP], BF16, tag="t2", name=f"t2{ph}")
                    # t2 = (out*r + bv) + x
                    _chained(nc.vector.scalar_tensor_tensor(
                        t2[:], tm[:], bv_sb[:, 0:1], xq_sb[:, bass.ts(ph, NQP)],
                        op0=mybir.AluOpType.add, op1=mybir.AluOpType.add,
                    ), dve_chain, "dve-order")
                    (nc.gpsimd if ph == 0 else nc.sync).dma_start(
                        out.ap()[:, bass.ts(ph, NQP)], t2[:]
                    )

                pending_epilogue = None
                for ph in range(2):
                    out_ph = pout.tile([C + 1, NQP], F32, tag="out",
                                       name=f"out{ph}")
                    e_cur = energy(ph, 0)
                    for c in range(NKC):
                        pt, is_i16 = do_exp(e_cur, ph, c)
                        if c + 1 < NKC:
                            e_cur = energy(ph, c + 1)
                        outs(out_ph, pt, is_i16, ph, c)
                        if c == 2 and pending_epilogue is not None:
                            epilogue(*pending_epilogue)
                            pending_epilogue = None
                    pending_epilogue = (ph, out_ph)
                epilogue(*pending_epilogue)

    if split:
        _split_sync_waits(nc)
    return nc


def host_prep(inputs):
    """Full inputs -> list of 8 per-core input maps."""
    x = np.asarray(inputs["x"], np.float32)
    wq = np.asarray(inputs["wq"], np.float32)
    bq = np.asarray(inputs["bq"], np.float32)
    wk = np.asarray(inputs["wk"], np.float32)
    wv = np.asarray(inputs["wv"], np.float32)
    bv = np.asarray(inputs["bv"], np.float32)
    gamma = np.asarray(inputs["gamma"], np.float32)

    bf = ml_dtypes.bfloat16
    f8 = ml_dtypes.float8_e4m3
    A = wq.T @ wk                     # (C, C): A[in_ch, out_ch]
    u = wk.T @ bq                     # (C,)
    gsc = float(gamma.reshape(-1)[0])

    # a_dr[p, i, s, mp] = fp8(S*A[2p+i, 2mp+s])
    a_dr = np.ascontiguousarray(
        (ESCALE * A).reshape(32, 2, 32, 2).transpose(0, 1, 3, 2)
    ).astype(f8)
    # u_sc[p, s] = S*u[2p+s]
    u_sc = np.ascontiguousarray((ESCALE * u).reshape(32, 2)).astype(np.float32)
    # wv_dr[p, i, c] = fp8(gamma*wv[c, 2p+i])
    wv_dr = np.ascontiguousarray(
        (gsc * wv.T).reshape(32, 2, C)
    ).astype(f8)
    bv_col = np.ascontiguousarray(gsc * bv[:, None]).astype(np.float32)

    xf = x.reshape(B, C, N)
    in_maps = []
    for core in range(N_CORES):
        b, h = core // 2, core % 2
        xb = xf[b]
        # roll the chunk axis so this core's query window is chunks 0..15
        xroll = np.roll(xb.reshape(C, NKC, 128), -16 * h, axis=1)
        x8 = xroll.astype(f8)  # [ch, c, m]
        # xk_f8[32b'+p, c, s, m] = x8[2p+s, c, m]
        band = np.ascontiguousarray(
            x8.reshape(32, 2, NKC, 128).transpose(0, 2, 1, 3)
        )
        xk_f8 = np.ascontiguousarray(np.concatenate([band, band], axis=0))
        xq = np.ascontiguousarray(
            xroll[:, :16, :].reshape(C, NQ).astype(bf)
        )
        in_maps.append(
            {
                "xk_f8": xk_f8,
                "a_dr": a_dr,
                "u_sc": u_sc,
                "wv_dr": wv_dr,
                "bv_col": bv_col,
                "xq_res": xq,
            }
        )
    return in_maps


_NC_CACHE = None


def kernel(**inputs) -> np.ndarray:
    global _NC_CACHE
    from concourse.bass_utils import run_bass_kernel_spmd

    if _NC_CACHE is None:
        _NC_CACHE = build_nc()
    nc = _NC_CACHE
    in_maps = host_prep(inputs)
    res = run_bass_kernel_spmd(nc, in_maps, core_ids=list(range(N_CORES)))
    x = np.asarray(inputs["x"], np.float32)
    full = np.empty((B, C, N), np.float32)
    for core in range(N_CORES):
        b, h = core // 2, core % 2
        full[b][:, h * NQ : (h + 1) * NQ] = np.asarray(
            res.results[core]["out"]
        ).astype(np.float32)
    return full.reshape(x.shape)


if __name__ == "__main__":
    rng = np.random.default_rng(0)
    demo = {
        "x": rng.standard_normal((B, C, 16, 16, 16), dtype=np.float32),
        "wq": 0.05 * rng.standard_normal((8, C), dtype=np.float32),
        "bq": 0.05 * rng.standard_normal((8,), dtype=np.float32),
        "wk": 0.05 * rng.standard_normal((8, C), dtype=np.float32),
        "bk": 0.05 * rng.standard_normal((8,), dtype=np.float32),
        "wv": 0.05 * rng.standard_normal((C, C), dtype=np.float32),
        "bv": 0.05 * rng.standard_normal((C,), dtype=np.float32),
        "gamma": np.ones((1,), np.float32),
    }
    print(kernel(**demo).shape)


# revision 48
# speedup vs baseline: 1.1708x; 1.1708x over previous
"""Trainium2 Bass kernel for nn_PamCell (spatial self-attention, B=4, C=64,
N=16^3=4096, CQ=8) on 8 NeuronCores.

Sharding: core i handles batch i//2 and query-half i%2 (2048 queries vs all
4096 keys). No collectives; host scatters inputs / gathers outputs.

Math: softmax rows are invariant to additive terms that depend only on the
query index, so with A = wq^T wk and u = wk^T bq,
    softmax(q k^T)[n, :] == softmax((A^T x_n + u) . x_m)[n, :]
which turns the QK contraction into a 64-dim contraction against the raw
input as keys. A is scaled by S=16 on the host (fp8e4m3 would denormalize
its ~0.007-magnitude entries) and descaled inside the exp.

Per-core pipeline (keys/queries in fp8e4m3; 2 query-phases of 1024):
  prologue: Q = S*(A^T x)+S*u via 4 fp8 window matmuls + DVE bias-cast to
            fp8; qb band 64-127 filled by an sbuf->sbuf DMA (partition
            remap); v^T per key chunk fp8, DVE copy to bf16.
  loop over 32 key chunks per phase:
      e[128,1024] = two fp8 matmuls (row bands alternate for ldweights
      overlap; the first phase-A chunks run same-band while the qb remap
      DMA completes)                                        (PE)
      p = exp(e/S): even chunks on ACT; odd on DVE as a Schraudolph exp2
          bit-trick (int16 affine, bitcast bf16)
      out[65,1024] += [v^T|1]^T @ p  (row 64 = denominator) (PE, bf16)
  epilogue per phase (overlapped into the next phase): r = 1/den via ACT
  Ln+Exp(-x) in bf16, ones-matmul broadcast to 64 partitions, ACT copy to
  SBUF, DVE mult + (add bv, add x) fused, bf16 DMA out. bv rides the
  epilogue because bv*den/den = bv.
"""

import sys

import numpy as np

try:
    import concourse.bass as bass
except ImportError:  # fresh interpreter without the env paths
    for _p in ("/root/.axon_site", "/root/.axon_site/_ro/trn_rl_repo",
               "/root/.axon_site/_ro/pypackages", "/opt/trn_rl_repo"):
        if _p not in sys.path:
            sys.path.append(_p)
    import concourse.bass as bass

import ml_dtypes

import concourse.tile as tile
from concourse import mybir
from concourse.vector_clock import ScopedClock

BF16 = mybir.dt.bfloat16
F32 = mybir.dt.float32
F8 = mybir.dt.float8e4
I16 = mybir.dt.int16
AF = mybir.ActivationFunctionType

B, C, N = 4, 64, 4096
NQ = N // 2          # queries per core
NKC = N // 128       # key chunks of 128
NQP = 1024           # queries per phase
N_CORES = 8

ESCALE = 16.0                       # host scale on A/u vs fp8 denormals
LOG2E = 1.4426950408889634
SCHRA_SCALE = 128.0 * LOG2E / ESCALE
SCHRA_BIAS = 16256.0


import os

K_NO_DVE = os.environ.get("K_NO_DVE") == "1"
K_NO_MIDLOOP = os.environ.get("K_NO_MIDLOOP") == "1"
K_SIMPLE_EPI = os.environ.get("K_SIMPLE_EPI") == "1"
K_NO_REMAP = os.environ.get("K_NO_REMAP") == "1"


def DVE_CHUNK(ph, c):
    # exp-offload set; phase A keeps the first chunks on ACT while the
    # DVE finishes the qb casts.
    if K_NO_DVE:
        return False
    return (c % 2 == 1) and (ph == 1 or c >= 5)


class _TileContextCompat(tile.TileContext):
    """Split the kernel-tail drain's sem waits across SP instructions;
    this walrus build allows only one sync-wait per CTRL instruction."""

    def _drain_and_barrier(self, tick_clock, wait_clock):
        probe = self.nc.sync.nop()
        wait_clock.add_sem_waits(
            probe.ins, ScopedClock({None: tick_clock.global_clock})
        )
        si = probe.ins.sync_info
        waits = list(si.on_wait) if si is not None else []
        if si is not None:
            probe.ins.sync_info = mybir.SyncInfo(
                on_wait=waits[:1], on_update=list(si.on_update)
            )
        for w in waits[1:]:
            nop = self.nc.sync.nop()
            nop.ins.sync_info = mybir.SyncInfo(on_wait=[w], on_update=[])

        self.nc.sync.drain()
        self.nc.all_engine_barrier()
        assert self.sems is not None
        popped = self.nc._tile_sem_poison_stack.pop()
        assert popped is self._sem_poison
        self.nc.clear_and_free_semaphores(list(self.sems.allocated().values()))
        self.nc.all_engine_barrier()


def _split_sync_waits(nc, max_waits=1):
    """This walrus build rejects instructions carrying more than one sync
    wait; hoist excess waits onto same-engine nops inserted just before."""
    for fn in nc.m.functions:
        for blk in fn.blocks:
            new = []
            changed = False
            for inst in blk.instructions:
                si = inst.sync_info
                if si is not None and si.on_wait and len(si.on_wait) > max_waits:
                    waits = list(si.on_wait)
                    excess = waits[:-max_waits]
                    for i in range(0, len(excess), max_waits):
                        nop = mybir.InstNoOp(
                            name=f"I-{nc.next_id()}-waitsplit", ins=[], outs=[]
                        )
                        nop.engine = inst.engine
                        nop.sync_info = mybir.SyncInfo(
                            on_wait=excess[i : i + max_waits], on_update=[]
                        )
                        new.append(nop)
                    inst.sync_info = mybir.SyncInfo(
                        on_wait=waits[-max_waits:], on_update=list(si.on_update)
                    )
                    changed = True
                new.append(inst)
            if changed:
                blk.instructions = new


def build_nc(split=True):
    nc = bass.Bass(
        "TRN2",
        target_bir_lowering=False,
        debug=False,
        enable_asserts=False,
    )
    # [64b+ch, c, m] = fp8(x[ch, 128c+m]); bands b=0,1 identical copies
    xk_f8 = nc.dram_tensor("xk_f8", (128, NKC, 128), F8, kind="ExternalInput")
    a8 = nc.dram_tensor("a8", (C, C), F8, kind="ExternalInput")  # S*A
    u_sc = nc.dram_tensor("u_sc", (C, 1), F32, kind="ExternalInput")  # S*u
    wv8 = nc.dram_tensor("wv8", (128, C), F8, kind="ExternalInput")  # dup
    bv_col = nc.dram_tensor("bv_col", (C, 1), F32, kind="ExternalInput")
    xq_res = nc.dram_tensor("xq_res", (C, NQ), BF16, kind="ExternalInput")
    out = nc.dram_tensor("out", (C, NQ), BF16, kind="ExternalOutput")

    with _TileContextCompat(nc) as tc:
        with tc.tile_pool(name="consts", bufs=1) as consts:
            xk8 = consts.tile([128, NKC, 128], F8, tag="xk8")
            a_sb = consts.tile([C, C], F8, tag="a_sb")
            u_sb = consts.tile([C, 1], F32, tag="u_sb")
            wv_sb = consts.tile([128, C], F8, tag="wv_sb")
            bv_sb = consts.tile([C, 1], F32, tag="bv_sb")
            xq_sb = consts.tile([C, NQ], BF16, tag="xq_sb")
            qb = consts.tile([128, NQ], F8, tag="qb")
            qstage = consts.tile([C, NQ], F8, tag="qstage")
            vt = consts.tile([128, NKC, C + 1], BF16, tag="vt")
            rln = consts.tile([1, NQP], F32, tag="rln")
            rec = consts.tile([1, NQ], BF16, tag="rec")
            ones_b = consts.tile([1, 128], BF16, tag="ones_b")
            warm_sb = consts.tile([1, 128], F32, tag="warm_sb")

            import bass_rust as _br

            pe_chain = [None]
            act_chain = [None]
            dve_chain = [None]

            def _chained(r, chain, reason="order"):
                if chain[0] is not None:
                    _br.add_dep_helper(r.ins, chain[0].ins, reason=reason)
                chain[0] = r
                return r

            nc.vector.memset(ones_b[:], 1.0)
            nc.gpsimd.memset(vt[:, :, C : C + 1], 1.0)
            # trigger the table load (natural_log set: Ln + Exp + Copy)
            _chained(nc.scalar.activation(warm_sb[:], ones_b[:], AF.Ln), act_chain)

            # ---- input DMAs ----
            # sync: smalls then fp8 keys band 0 (chunk-major, so arrival
            # outpaces the loop's 1 chunk/us). gpsimd: band 1. scalar: the
            # qb partition-remap (waits on casts), then the bf16 residual
            # (epilogue-only), then output DMAs.
            nc.sync.dma_start(a_sb[:], a8.ap())
            nc.sync.dma_start(u_sb[:], u_sc.ap())
            nc.sync.dma_start(wv_sb[:], wv8.ap())
            nc.sync.dma_start(bv_sb[:], bv_col.ap())
            for band, eng in ((0, nc.sync), (1, nc.gpsimd)):
                for lo, hi in ((0, 8), (8, 16), (16, 32)):
                    eng.dma_start(
                        xk8[bass.ts(band, 64), lo:hi, :],
                        xk_f8.ap()[bass.ts(band, 64), lo:hi, :],
                    )

            # ---- prologue ----
            # Q: q32[ch, g*512+j] = S*(A^T x)[ch, ...] via fp8 window
            # matmuls straight off the key tensor (queries are chunks
            # 0..15 in the host-rolled order). Phase B's Q is emitted
            # mid-loop (emit_q(1)) so its later DMA pieces don't stall
            # the in-order PE queue before the first energy.
            def emit_q(ph, q32):
                for g in range(2):
                    cw = 8 * ph + 4 * g
                    _chained(nc.tensor.matmul(
                        q32[:C, bass.ts(g, 512)],
                        a_sb[:],
                        xk8[0:64, cw : cw + 4, :].rearrange(
                            "p a b -> p (a b)"),
                        start=True, stop=True,
                        skip_group_check=True,
                    ), pe_chain, "pe-order")
                for g in range(2):
                    _chained(nc.vector.tensor_scalar(
                        qb[0:64, bass.ds(NQP * ph + 512 * g, 512)],
                        q32[:C, bass.ts(g, 512)],
                        u_sb[:, 0:1], None,
                        op0=mybir.AluOpType.add,
                    ), dve_chain, "dve-order")
                    # second copy for the band 64-127 remap DMA (a direct
                    # qb->qb self-copy deadlocks the queue)
                    _chained(nc.vector.tensor_scalar(
                        qstage[0:64, bass.ds(NQP * ph + 512 * g, 512)],
                        q32[:C, bass.ts(g, 512)],
                        u_sb[:, 0:1], None,
                        op0=mybir.AluOpType.add,
                    ), dve_chain, "dve-order")
                # band 64-127 copy: only DMA can remap partitions
                if not K_NO_REMAP:
                    nc.scalar.dma_start(
                        qb[64:128, bass.ts(ph, NQP)],
                        qstage[0:64, bass.ts(ph, NQP)],
                    )

            # v^T group: 4 chunk matmuls (fp8, band-alternated) into a psum
            # view + bf16 copy on DVE. Groups 0-1 run in the prologue;
            # 2-7 are emitted mid-loop (their key chunks DMA-arrive late,
            # and unchained PE work can deadlock against the DVE chain).
            def emit_vt(grp, vp):
                for k in range(4):
                    c = 4 * grp + k
                    # all band 0: short fp8 matmuls alternating row bands
                    # crash the PE (NRT_EXEC_UNIT_UNRECOVERABLE)
                    _chained(nc.tensor.matmul(
                        vp[:, k, :],
                        xk8[0:64, c, :],
                        wv_sb[0:64, :],
                        start=True, stop=True, skip_group_check=True,
                    ), pe_chain, "pe-order")
                _chained(nc.vector.tensor_copy(
                    vt[:, bass.ts(grp, 4), :C], vp[:]
                ), dve_chain, "dve-order")

            with tc.tile_pool(name="psum_pro", bufs=1, space="PSUM") as pro:
                q32a = pro.tile([C, NQP], F32, tag="q32", bufs=2,
                                name="q32_0")
                emit_q(0, q32a)
                n_pro_vt = NKC // 4 if K_NO_MIDLOOP else 2
                for grp in range(n_pro_vt):
                    vp = pro.tile([128, 4, C], F32, tag="vp", bufs=2,
                                  name=f"vp{grp}")
                    emit_vt(grp, vp)
                if K_NO_MIDLOOP:
                    q32b = pro.tile([C, NQP], F32, tag="q32", bufs=2,
                                    name="q32_1")
                    emit_q(1, q32b)

            # residual DMA after the phase-A remap on the scalar queue
            # (needed from the phase-A epilogue onward)
            nc.scalar.dma_start(xq_sb[:], xq_res.ap())

            # ---- main loop ----
            with (
                tc.tile_pool(name="psum_e", bufs=2, space="PSUM") as pe_pool,
                tc.tile_pool(name="psum_out", bufs=2, space="PSUM") as pout,
                tc.tile_pool(name="ptb_pool", bufs=3) as ptb_pool,
                tc.tile_pool(name="pti_pool", bufs=2) as pti_pool,
                tc.tile_pool(name="epi_pool", bufs=2) as epi_pool,
            ):
                def energy(ph, c):
                    e = pe_pool.tile([128, 1024], F32, tag="e", name=f"e{ph}_{c}")
                    for g in range(2):
                        # first phase-A chunks run both groups on band 0
                        # while the qb band-remap DMA is in flight
                        b = 64 * g if (ph, c) >= (0, 4) else 0
                        if K_NO_REMAP:
                            b = 0
                        _chained(nc.tensor.matmul(
                            e[:, bass.ts(g, 512)],
                            xk8[b : b + 64, c, :],
                            qb[b : b + 64, bass.ds(NQP * ph + 512 * g, 512)],
                            start=True, stop=True,
                        ), pe_chain, "pe-order")
                    return e

                def do_exp(e, ph, c):
                    if DVE_CHUNK(ph, c):
                        pt = pti_pool.tile([128, 1024], I16, tag="pti",
                                           name=f"pti{ph}_{c}")
                        _chained(nc.vector.tensor_scalar(
                            pt[:], e[:], SCHRA_SCALE, SCHRA_BIAS,
                            op0=mybir.AluOpType.mult, op1=mybir.AluOpType.add,
                        ), dve_chain, "dve-order")
                        return pt, True
                    pt = ptb_pool.tile([128, 1024], BF16, tag="ptb",
                                       name=f"ptb{ph}_{c}")
                    _chained(nc.scalar.activation(
                        pt[:], e[:], AF.Exp, scale=1.0 / ESCALE,
                    ), act_chain, "act-order")
                    return pt, False

                def outs(out_ph, pt, is_i16, ph, c):
                    for g in range(2):
                        rhs = pt[:, bass.ts(g, 512)]
                        if is_i16:
                            rhs = rhs.bitcast(BF16)
                        _chained(nc.tensor.matmul(
                            out_ph[:, bass.ts(g, 512)],
                            vt[:, c, :],
                            rhs,
                            start=(c == 0), stop=(c == NKC - 1),
                            skip_group_check=True,
                        ), pe_chain, "pe-order")

                def epilogue(ph, out_ph):
                    if K_SIMPLE_EPI:
                        t2s = epi_pool.tile([C, NQP], BF16, tag="t2s",
                                            name=f"t2s{ph}")
                        _chained(nc.vector.tensor_copy(
                            t2s[:], out_ph[:C, :]), dve_chain, "dve-order")
                        (nc.sync if ph == 0 else nc.scalar).dma_start(
                            out.ap()[:, bass.ts(ph, NQP)], t2s[:])
                        return
                    # r = 1/den = exp(-ln(den)) on ACT (bf16 out); K=1
                    # ones-matmul broadcast into an e-pool psum slot; ACT
                    # copy to SBUF (tensor_tensor cannot read two PSUMs);
                    # DVE mult then fused (+bv, +x); bf16 DMA out.
                    _chained(nc.scalar.activation(
                        rln[:], out_ph[C : C + 1, :], AF.Ln,
                    ), act_chain, "act-order")
                    with nc.allow_low_precision(reason="1/den fits bf16"):
                        _chained(nc.scalar.activation(
                            rec[:, bass.ts(ph, NQP)], rln[:], AF.Exp,
                            scale=-1.0,
                        ), act_chain, "act-order")
                    bce = pe_pool.tile([128, 1024], F32, tag="e", name=f"bc{ph}")
                    for g in range(2):
                        _chained(nc.tensor.matmul(
                            bce[:C, bass.ts(g, 512)],
                            ones_b[:, :C],
                            rec[:, bass.ds(NQP * ph + 512 * g, 512)],
                            start=True, stop=True, skip_group_check=True,
                        ), pe_chain, "pe-order")
                    bc_sb = epi_pool.tile([C, NQP], F32, tag="bc_sb",
                                          name=f"bc_sb{ph}")
                    _chained(nc.scalar.copy(bc_sb[:], bce[:C, :]),
                             act_chain, "act-order")
                    for h in range(2):  # halves pipeline the tail
                        hs = bass.ts(h, 512)
                        tm = epi_pool.tile([C, 512], F32, tag=f"tm{h}",
                                           name=f"tm{ph}_{h}")
                        _chained(nc.vector.tensor_tensor(
                            tm[:], out_ph[:C, hs], bc_sb[:, hs],
                            mybir.AluOpType.mult,
                        ), dve_chain, "dve-order")
                        t2 = epi_pool.tile([C, 512], BF16, tag=f"t2{h}",
                                           name=f"t2{ph}_{h}")
                        _chained(nc.vector.scalar_tensor_tensor(
                            t2[:], tm[:], bv_sb[:, 0:1],
                            xq_sb[:, bass.ds(NQP * ph + 512 * h, 512)],
                            op0=mybir.AluOpType.add, op1=mybir.AluOpType.add,
                        ), dve_chain, "dve-order")
                        (nc.sync if ph == 0 else nc.scalar).dma_start(
                            out.ap()[:, bass.ds(NQP * ph + 512 * h, 512)],
                            t2[:],
                        )

                pending_epilogue = None
                for ph in range(2):
                    out_ph = pout.tile([C + 1, NQP], F32, tag="out",
                                       name=f"out{ph}")
                    e_cur = energy(ph, 0)
                    for c in range(NKC):
                        pt, is_i16 = do_exp(e_cur, ph, c)
                        if c + 1 < NKC:
                            e_cur = energy(ph, c + 1)
                        outs(out_ph, pt, is_i16, ph, c)
                        if (not K_NO_MIDLOOP and ph == 0 and c >= 4
                                and c <= 14 and c % 2 == 0):
                            # vt groups 2-7 ride e-pool slots mid-loop
                            grp = c // 2
                            ev = pe_pool.tile([128, 1024], F32, tag="e",
                                              name=f"vp{grp}")
                            emit_vt(grp, ev[:, 0:256].rearrange(
                                "p (a b) -> p a b", a=4))
                        if not K_NO_MIDLOOP and ph == 0 and c == 7:
                            # phase-B Q, off the loop-start critical path;
                            # its psum rides an e-pool slot
                            q32b = pe_pool.tile([128, 1024], F32, tag="e",
                                                name="q32_1")
                            emit_q(1, q32b)
                        if c == 2 and pending_epilogue is not None:
                            epilogue(*pending_epilogue)
                            pending_epilogue = None
                    pending_epilogue = (ph, out_ph)
                epilogue(*pending_epilogue)

    if split:
        _split_sync_waits(nc)
    return nc


def host_prep(inputs):
    """Full inputs -> list of 8 per-core input maps."""
    x = np.asarray(inputs["x"], np.float32)
    wq = np.asarray(inputs["wq"], np.float32)
    bq = np.asarray(inputs["bq"], np.float32)
    wk = np.asarray(inputs["wk"], np.float32)
    wv = np.asarray(inputs["wv"], np.float32)
    bv = np.asarray(inputs["bv"], np.float32)
    gamma = np.asarray(inputs["gamma"], np.float32)

    bf = ml_dtypes.bfloat16
    f8 = ml_dtypes.float8_e4m3
    A = wq.T @ wk                     # (C, C): A[in_ch, out_ch]
    u = wk.T @ bq                     # (C,)
    gsc = float(gamma.reshape(-1)[0])

    a8 = np.ascontiguousarray(ESCALE * A).astype(f8)
    u_sc = np.ascontiguousarray((ESCALE * u)[:, None]).astype(np.float32)
    wv8_1 = (gsc * wv.T).astype(f8)   # [in_ch, c]
    wv8 = np.ascontiguousarray(np.concatenate([wv8_1, wv8_1], axis=0))
    bv_col = np.ascontiguousarray(gsc * bv[:, None]).astype(np.float32)

    xf = x.reshape(B, C, N)
    in_maps = []
    for core in range(N_CORES):
        b, h = core // 2, core % 2
        xb = xf[b]
        # roll the chunk axis so this core's query window is chunks 0..15
        xroll = np.roll(xb.reshape(C, NKC, 128), -16 * h, axis=1)
        band = xroll.astype(f8)
        xk_f8 = np.ascontiguousarray(np.concatenate([band, band], axis=0))
        xq = np.ascontiguousarray(
            xroll[:, :16, :].reshape(C, NQ).astype(bf)
        )
        in_maps.append(
            {
                "xk_f8": xk_f8,
                "a8": a8,
                "u_sc": u_sc,
                "wv8": wv8,
                "bv_col": bv_col,
                "xq_res": xq,
            }
        )
    return in_maps


_NC_CACHE = None


def kernel(**inputs) -> np.ndarray:
    global _NC_CACHE
    from concourse.bass_utils import run_bass_kernel_spmd

    if _NC_CACHE is None:
        _NC_CACHE = build_nc()
    nc = _NC_CACHE
    in_maps = host_prep(inputs)
    res = run_bass_kernel_spmd(nc, in_maps, core_ids=list(range(N_CORES)))
    x = np.asarray(inputs["x"], np.float32)
    full = np.empty((B, C, N), np.float32)
    for core in range(N_CORES):
        b, h = core // 2, core % 2
        full[b][:, h * NQ : (h + 1) * NQ] = np.asarray(
            res.results[core]["out"]
        ).astype(np.float32)
    return full.reshape(x.shape)


if __name__ == "__main__":
    rng = np.random.default_rng(0)
    demo = {
        "x": rng.standard_normal((B, C, 16, 16, 16), dtype=np.float32),
        "wq": 0.05 * rng.standard_normal((8, C), dtype=np.float32),
        "bq": 0.05 * rng.standard_normal((8,), dtype=np.float32),
        "wk": 0.05 * rng.standard_normal((8, C), dtype=np.float32),
        "bk": 0.05 * rng.standard_normal((8,), dtype=np.float32),
        "wv": 0.05 * rng.standard_normal((C, C), dtype=np.float32),
        "bv": 0.05 * rng.standard_normal((C,), dtype=np.float32),
        "gamma": np.ones((1,), np.float32),
    }
    print(kernel(**demo).shape)
